# revision 26
# baseline (speedup 1.0000x reference)
"""Trainium2 Bass kernel for DigitConvolutionalModel.

Computation: x[B,784] -> reshape [28,28] -> 3x3 valid conv (single channel)
-> relu -> flatten [676] -> linear to 10 classes.

Strategy (pure data parallel over 8 cores, batch-sharded; per core 4096 rows
in 8 macro-tiles of 512):
  * Host prep: x is cast to bf16 and zero-padded 784 -> 896 columns (7*128).
    The conv is a banded matmul h[o, b] = sum_p Wband[p, o] x^T[p, b] with
    o = 28*oi + oj embedded in 768 slots; Wband bandwidth 59 means each
    128-wide o-chunk needs two 128-row pixel blocks -> 12 conv matmuls per
    macro (all operands bf16, fp32 PSUM accumulation).
  * x^T lands in SBUF directly via ONE DMA-xbar transpose instruction per
    macro: [512, 896] bf16 DRAM -> [128, 7, 512] SBUF (pixel-major chunks).
    No separate load, no tail handling (the pad covers pixels 768..783 that
    o-chunk 5 needs; pad columns have zero weights).
  * ReLU + f32->bf16 cast happens in the PSUM->SBUF drain, round-robined
    across the Activation / DVE / Pool engines; the FC layer is 6
    accumulating matmuls with the embedded fc weight as the stationary
    operand -> out^T [10, b] in PSUM.  Bias is added via tensor_scalar on
    DVE, small PE transposes bring the result back to batch-major, and one
    DMA per macro stores [512, 10] contiguously.
  * After Tile scheduling, a post-pass hoists excess per-instruction
    semaphore waits into standalone EventSemaphore instructions (this
    walrus build only accepts one sync-wait command per instruction).
"""

import sys

for _p in ("/opt/trn_rl_repo",):
    if _p not in sys.path:
        sys.path.insert(0, _p)

import ml_dtypes
import numpy as np

import concourse.bass as bass
import concourse.mybir as mybir
from concourse.bass_utils import run_bass_kernel_spmd
from concourse.tile import TileContext

B = 32768
PIX = 784  # 28*28
PPAD = 896  # 7*128; zero-padded pixel count
EMB = 768  # 6*128; embedded conv-output length (o = 28*oi + oj, max 725)
NCLS = 10
NCORES = 8
BL = B // NCORES  # rows per core
MACRO = 512  # batch rows processed per macro-tile
_BF16 = ml_dtypes.bfloat16


def _host_packs(conv_w, fc_w, fc_b):
    """Pack all constants into three arrays (three DMAs).

    wpack [128, 1596] bf16 = w1 blocks (6x128 cols) | w2 blocks (6x128)
                             | fc chunks (6x10)
    fpack [10, 10] f32     = eye10
    vpack [128, 1] f32     = ones-row selector: 1.0 at row 87, else 0

    The fc bias rides the matmul: embedded slot o=727 (invalid conv output,
    = row 87 of chunk 5) gets fct[727] = fc_b, and the chunk-5 ReLU drain
    adds vpack so ht[87] == 1.
    """
    oi = np.repeat(np.arange(26), 26)
    oj = np.tile(np.arange(26), 26)
    o = 28 * oi + oj  # embedded output index (0..725)
    wband = np.zeros((PPAD, EMB), np.float32)
    for ki in range(3):
        for kj in range(3):
            wband[o + 28 * ki + kj, o] = conv_w[ki, kj]
    cols = [wband[128 * q : 128 * (q + 1), 128 * q : 128 * (q + 1)] for q in range(6)]
    cols += [wband[128 * (q + 1) : 128 * (q + 2), 128 * q : 128 * (q + 1)] for q in range(6)]
    fct = np.zeros((EMB, NCLS), np.float32)
    fct[o, :] = fc_w[:, 26 * oi + oj].T
    fct[727, :] = fc_b
    cols += [fct[128 * q : 128 * (q + 1)] for q in range(6)]
    wpack = np.ascontiguousarray(np.concatenate(cols, axis=1)).astype(_BF16)
    fpack = np.ascontiguousarray(np.eye(NCLS, dtype=np.float32))
    vpack = np.zeros((128, 1), np.float32)
    vpack[727 - 640, 0] = 1.0
    return wpack, fpack, vpack


def _host_x(x):
    """bf16-cast and zero-pad x to [B, 896]."""
    xb = np.zeros((x.shape[0], PPAD), dtype=_BF16)
    xb[:, :PIX] = x.astype(_BF16)
    return xb


def _split_waits(nc, max_waits=1):
    """Hoist excess semaphore waits into standalone EventSemaphore
    instructions (walrus codegen accepts one sync-wait command per
    instruction; the Tile scheduler can attach more)."""
    for f in nc.m.functions:
        for blk in f.blocks:
            new = []
            changed = False
            for inst in blk.instructions:
                si = inst.sync_info
                if si is not None and len(si.on_wait) > max_waits:
                    waits = list(si.on_wait)
                    excess, keep = waits[:-max_waits], waits[-max_waits:]
                    for i, w in enumerate(excess):
                        new.append(
                            mybir.InstEventSemaphore(
                                name=f"{inst.name}-prewait{i}",
                                engine=inst.engine,
                                ins=[],
                                outs=[],
                                sync_info=mybir.SyncInfo(on_wait=[w], on_update=[]),
                            )
                        )
                    inst.sync_info = mybir.SyncInfo(
                        on_wait=keep, on_update=list(si.on_update)
                    )
                    changed = True
                new.append(inst)
            if changed:
                blk.instructions = new


# Tunables.
CFG = {
    "xt_bufs": 6,
    "ht_bufs": 13,
    "hp_bufs": 4,
    "tp_bufs": 2,
    "op_bufs": 2,
    "osb_bufs": 3,
    # 0=ACT 1=DVE per conv chunk (Pool/GPSIMD cannot read PSUM on TRN2 HW);
    # q=5 always runs the DVE tensor_scalar that injects the bias ones-row.
    "drain_order": (0, 1, 0, 1, 0, 1),
    "drain_out": 1,  # engine for the ops PSUM->SBUF copy (0=ACT 1=DVE)
    # Macro-tile batch sizes: small tiles at the start fill the DMA->PE
    # pipeline sooner (first conv waits on only a 128-row transpose), small
    # tiles at the end shorten the drain->fc->bias->store tail.
    "macros": (128, 128, 256, 512, 512, 512, 512, 512, 512, 256, 128, 128),
    "warmup_pe": 3,  # dependency-free PE matmuls at t~0 (p-state ramp)
    "pre_xpose": 2,  # input transposes emitted before the small const loads
}

_DR_RELU = mybir.ActivationFunctionType.Relu


def build_nc(bl=BL, split_waits=True):
    nc = bass.Bass("TRN2")
    xb = nc.dram_tensor("xb", [bl, PPAD], mybir.dt.bfloat16, kind="ExternalInput")
    wpk = nc.dram_tensor("wpack", [128, 1596], mybir.dt.bfloat16, kind="ExternalInput")
    fpk = nc.dram_tensor("fpack", [NCLS, NCLS], mybir.dt.float32, kind="ExternalInput")
    vpk = nc.dram_tensor("vpack", [128, 1], mybir.dt.float32, kind="ExternalInput")
    out = nc.dram_tensor("out", [bl, NCLS], mybir.dt.float32, kind="ExternalOutput")

    macros = [s for s in CFG["macros"]]
    assert sum(macros) * (bl // bl) == bl if bl == BL else True
    if sum(macros) != bl:  # fall back to uniform tiling for other shard sizes
        macros = [MACRO] * (bl // MACRO)

    with TileContext(nc) as tc:
        with (
            tc.tile_pool(name="const", bufs=1) as cp,
            tc.tile_pool(name="xt", bufs=CFG["xt_bufs"]) as xtp,
            tc.tile_pool(name="ht", bufs=CFG["ht_bufs"]) as htp,
            tc.tile_pool(name="osb", bufs=CFG["osb_bufs"]) as osp,
            tc.tile_pool(name="hps", bufs=CFG["hp_bufs"], space="PSUM") as hpp,
            tc.tile_pool(name="tps", bufs=CFG["tp_bufs"], space="PSUM") as tpp,
            tc.tile_pool(name="ops", bufs=CFG["op_bufs"], space="PSUM") as opp,
        ):
            # PE warmup: matmuls on a zeroed tile, dependency-free so they
            # issue immediately and start the p-state ramp clock.
            if CFG["warmup_pe"]:
                zt = cp.tile([128, 128], mybir.dt.bfloat16)
                nc.gpsimd.memset(zt[:, :], 0)
                for w in range(CFG["warmup_pe"]):
                    wtp = tpp.tile([128, NCLS], mybir.dt.float32, tag="tp")
                    nc.tensor.matmul(
                        wtp[:, :], zt[:, :], zt[:, 0:NCLS], start=True, stop=True
                    )

            wps = cp.tile([128, 1596], mybir.dt.bfloat16)
            nc.sync.dma_start(wps[:, :], wpk[:, :])
            w1s = wps[:, 0:768]
            w2s = wps[:, 768:1536]
            fcs = wps[:, 1536:1596]

            # First input transposes go ahead of the small const loads so the
            # conv pipeline fills as early as possible.
            starts = []
            acc = 0
            for nb in macros:
                starts.append(acc)
                acc += nb
            # xt tiles are allocated at the exact macro size so the xbar
            # transpose's destination AP is fully contiguous (a strided 3D
            # dest was flakily mis-written by the hardware xbar).
            def emit_xpose(pm):
                nb = macros[pm]
                xt = xtp.tile([128, 7, nb], mybir.dt.bfloat16, tag=f"xt{nb}")
                nc.sync.dma_start_transpose(
                    xt[:, :, :], xb[starts[pm] : starts[pm] + nb, :]
                )
                return xt

            pre_xt = {}
            for pm in range(min(CFG["pre_xpose"], len(macros))):
                pre_xt[pm] = emit_xpose(pm)

            fps = cp.tile([NCLS, NCLS], mybir.dt.float32)
            nc.sync.dma_start(fps[:, :], fpk[:, :])
            vps = cp.tile([128, 1], mybir.dt.float32)
            nc.sync.dma_start(vps[:, :], vpk[:, :])
            eyfs = fps[:, :]

            def emit_conv(m, nb, xt):
                """12 conv matmuls + 6 relu drains; returns ht tiles."""
                hts = []
                for q in range(6):
                    hp = hpp.tile([128, MACRO], mybir.dt.float32, tag="hp")
                    nc.tensor.matmul(
                        hp[:, 0:nb], w1s[:, 128 * q : 128 * (q + 1)], xt[:, q, :],
                        start=True, stop=False,
                    )
                    nc.tensor.matmul(
                        hp[:, 0:nb], w2s[:, 128 * q : 128 * (q + 1)], xt[:, q + 1, :],
                        start=False, stop=True,
                    )
                    ht = htp.tile([128, MACRO], mybir.dt.bfloat16, tag="ht")
                    if q == 5:
                        # relu + ones-row injection (row 87 <- 0 + 1.0) so the
                        # fc matmul's fct[727] row adds the bias.
                        nc.vector.tensor_scalar(
                            ht[:, 0:nb], hp[:, 0:nb], 0.0, vps[:, :],
                            op0=mybir.AluOpType.max, op1=mybir.AluOpType.add,
                        )
                    elif CFG["drain_order"][q] == 0:
                        nc.scalar.activation(ht[:, 0:nb], hp[:, 0:nb], _DR_RELU)
                    else:
                        nc.vector.tensor_scalar_max(ht[:, 0:nb], hp[:, 0:nb], 0.0)
                    hts.append(ht)
                return hts

            def emit_fc(nb, hts):
                """6 accumulating fc matmuls + PSUM->SBUF copy of out^T."""
                ops = opp.tile([NCLS, MACRO], mybir.dt.float32, tag="ops")
                for q in range(6):
                    nc.tensor.matmul(
                        ops[:, 0:nb], fcs[:, NCLS * q : NCLS * (q + 1)],
                        hts[q][:, 0:nb], start=(q == 0), stop=(q == 5),
                    )
                ot = osp.tile([NCLS, MACRO], mybir.dt.float32, tag="ot")
                if CFG["drain_out"] == 0:
                    nc.scalar.copy(ot[:, 0:nb], ops[:, 0:nb])
                else:
                    nc.vector.tensor_copy(ot[:, 0:nb], ops[:, 0:nb])
                return ot

            def emit_out(m, nb, ot):
                """PE transposes back to batch-major + one store DMA."""
                r0 = starts[m]
                nbc = nb // 128
                ob4 = osp.tile([128, (MACRO // 128) * NCLS], mybir.dt.float32, tag="ob4")
                for bc in range(nbc):
                    op2 = tpp.tile([128, NCLS], mybir.dt.float32, tag="tp")
                    nc.tensor.transpose(
                        op2[:, :], ot[:, bc * 128 : (bc + 1) * 128], eyfs[:, :]
                    )
                    if bc % 2 == 0:
                        nc.vector.tensor_copy(ob4[:, bc * NCLS : (bc + 1) * NCLS], op2[:, :])
                    else:
                        nc.scalar.copy(ob4[:, bc * NCLS : (bc + 1) * NCLS], op2[:, :])
                nc.sync.dma_start(
                    out[r0 : r0 + nb, :].rearrange("(b p) c -> p b c", p=128),
                    ob4[:, 0 : nbc * NCLS].rearrange("p (b c) -> p b c", c=NCLS),
                )

            # Two-deep software pipeline on the in-order PE queue: iteration m
            # emits conv_m, fc_{m-1}, out_{m-2}, so fc never waits on a drain
            # that was just issued and out-transposes never wait on the ot copy.
            fcq = []   # [(m, nb, hts)]
            outq = []  # [(m, nb, ot)]
            for m, nb in enumerate(macros):
                xt = pre_xt.pop(m) if m in pre_xt else emit_xpose(m)
                hts = emit_conv(m, nb, xt)
                fcq.append((m, nb, hts))
                if len(fcq) > 1:
                    fm, fnb, fhts = fcq.pop(0)
                    outq.append((fm, fnb, emit_fc(fnb, fhts)))
                if len(outq) > 1:
                    om, onb, oot = outq.pop(0)
                    emit_out(om, onb, oot)
            while fcq:
                fm, fnb, fhts = fcq.pop(0)
                outq.append((fm, fnb, emit_fc(fnb, fhts)))
            while outq:
                om, onb, oot = outq.pop(0)
                emit_out(om, onb, oot)
    if split_waits:
        _split_waits(nc)
    return nc


_CACHED = {}


def _get_nc(bl):
    if bl not in _CACHED:
        _CACHED[bl] = build_nc(bl)
    return _CACHED[bl]


def kernel(x, conv_w, fc_w, fc_b):
    x = np.ascontiguousarray(np.asarray(x, dtype=np.float32))
    conv_w = np.asarray(conv_w, dtype=np.float32)
    fc_w = np.asarray(fc_w, dtype=np.float32)
    fc_b = np.asarray(fc_b, dtype=np.float32)

    wpack, fpack, vpack = _host_packs(conv_w, fc_w, fc_b)
    xbig = _host_x(x)

    nc = _get_nc(BL)
    in_maps = []
    for c in range(NCORES):
        in_maps.append(
            {
                "xb": xbig[c * BL : (c + 1) * BL],
                "wpack": wpack,
                "fpack": fpack,
                "vpack": vpack,
            }
        )
    # The axon-proxied NeuronCores occasionally come up wedged
    # (NRT_EXEC_UNIT_UNRECOVERABLE) on the first execute after idle periods;
    # a retry on a fresh execute reliably recovers.
    last_err = None
    for _attempt in range(3):
        try:
            res = run_bass_kernel_spmd(nc, in_maps, core_ids=list(range(NCORES)))
            break
        except Exception as e:  # noqa: BLE001
            last_err = e
            if "UNRECOVERABLE" not in str(e) and "desynced" not in str(e):
                raise
    else:
        raise last_err
    out = np.concatenate([np.asarray(r["out"]) for r in res.results], axis=0)
    return out


if __name__ == "__main__":
    rng = np.random.default_rng(0)
    xs = rng.standard_normal((B, PIX), dtype=np.float32)
    cw = rng.standard_normal((3, 3), dtype=np.float32)
    fw = (rng.standard_normal((NCLS, 676)) * 0.05).astype(np.float32)
    fb = (rng.standard_normal((NCLS,)) * 0.05).astype(np.float32)
    res = kernel(xs, cw, fw, fb)
    print(res.shape, res.dtype)


# revision 29
# speedup vs baseline: 1.0198x; 1.0198x over previous
"""Trainium2 Bass kernel for DigitConvolutionalModel.

Computation: x[B,784] -> reshape [28,28] -> 3x3 valid conv (single channel)
-> relu -> flatten [676] -> linear to 10 classes.

Strategy (pure data parallel over 8 cores, batch-sharded; per core 4096 rows
in 8 macro-tiles of 512):
  * Host prep: x is cast to bf16 and zero-padded 784 -> 896 columns (7*128).
    The conv is a banded matmul h[o, b] = sum_p Wband[p, o] x^T[p, b] with
    o = 28*oi + oj embedded in 768 slots; Wband bandwidth 59 means each
    128-wide o-chunk needs two 128-row pixel blocks -> 12 conv matmuls per
    macro (all operands bf16, fp32 PSUM accumulation).
  * x^T lands in SBUF directly via ONE DMA-xbar transpose instruction per
    macro: [512, 896] bf16 DRAM -> [128, 7, 512] SBUF (pixel-major chunks).
    No separate load, no tail handling (the pad covers pixels 768..783 that
    o-chunk 5 needs; pad columns have zero weights).
  * ReLU + f32->bf16 cast happens in the PSUM->SBUF drain, round-robined
    across the Activation / DVE / Pool engines; the FC layer is 6
    accumulating matmuls with the embedded fc weight as the stationary
    operand -> out^T [10, b] in PSUM.  Bias is added via tensor_scalar on
    DVE, small PE transposes bring the result back to batch-major, and one
    DMA per macro stores [512, 10] contiguously.
  * After Tile scheduling, a post-pass hoists excess per-instruction
    semaphore waits into standalone EventSemaphore instructions (this
    walrus build only accepts one sync-wait command per instruction).
"""

import sys

for _p in ("/opt/trn_rl_repo",):
    if _p not in sys.path:
        sys.path.insert(0, _p)

import ml_dtypes
import numpy as np

import concourse.bass as bass
import concourse.mybir as mybir
from concourse.bass_utils import run_bass_kernel_spmd
from concourse.tile import TileContext

B = 32768
PIX = 784  # 28*28
PPAD = 896  # 7*128; zero-padded pixel count
EMB = 768  # 6*128; embedded conv-output length (o = 28*oi + oj, max 725)
NCLS = 10
NCORES = 8
BL = B // NCORES  # rows per core
MACRO = 512  # batch rows processed per macro-tile
_BF16 = ml_dtypes.bfloat16


def _host_packs(conv_w, fc_w, fc_b):
    """Pack all constants into three arrays (three DMAs).

    wpack [128, 1596] bf16 = w1 blocks (6x128 cols) | w2 blocks (6x128)
                             | fc chunks (6x10)
    fpack [10, 10] f32     = eye10
    vpack [128, 1] f32     = ones-row selector: 1.0 at row 87, else 0

    The fc bias rides the matmul: embedded slot o=727 (invalid conv output,
    = row 87 of chunk 5) gets fct[727] = fc_b, and the chunk-5 ReLU drain
    adds vpack so ht[87] == 1.
    """
    oi = np.repeat(np.arange(26), 26)
    oj = np.tile(np.arange(26), 26)
    o = 28 * oi + oj  # embedded output index (0..725)
    wband = np.zeros((PPAD, EMB), np.float32)
    for ki in range(3):
        for kj in range(3):
            wband[o + 28 * ki + kj, o] = conv_w[ki, kj]
    cols = [wband[128 * q : 128 * (q + 1), 128 * q : 128 * (q + 1)] for q in range(6)]
    cols += [wband[128 * (q + 1) : 128 * (q + 2), 128 * q : 128 * (q + 1)] for q in range(6)]
    fct = np.zeros((EMB, NCLS), np.float32)
    fct[o, :] = fc_w[:, 26 * oi + oj].T
    fct[727, :] = fc_b
    cols += [fct[128 * q : 128 * (q + 1)] for q in range(6)]
    wpack = np.ascontiguousarray(np.concatenate(cols, axis=1)).astype(_BF16)
    fpack = np.ascontiguousarray(np.eye(NCLS, dtype=np.float32))
    vpack = np.zeros((128, 1), np.float32)
    vpack[727 - 640, 0] = 1.0
    return wpack, fpack, vpack


def _host_x(x):
    """bf16-cast and zero-pad x to [B, 896]."""
    xb = np.zeros((x.shape[0], PPAD), dtype=_BF16)
    xb[:, :PIX] = x.astype(_BF16)
    return xb


def _split_waits(nc, max_waits=1):
    """Hoist excess semaphore waits into standalone EventSemaphore
    instructions (walrus codegen accepts one sync-wait command per
    instruction; the Tile scheduler can attach more)."""
    for f in nc.m.functions:
        for blk in f.blocks:
            new = []
            changed = False
            for inst in blk.instructions:
                si = inst.sync_info
                if si is not None and len(si.on_wait) > max_waits:
                    waits = list(si.on_wait)
                    excess, keep = waits[:-max_waits], waits[-max_waits:]
                    for i, w in enumerate(excess):
                        new.append(
                            mybir.InstEventSemaphore(
                                name=f"{inst.name}-prewait{i}",
                                engine=inst.engine,
                                ins=[],
                                outs=[],
                                sync_info=mybir.SyncInfo(on_wait=[w], on_update=[]),
                            )
                        )
                    inst.sync_info = mybir.SyncInfo(
                        on_wait=keep, on_update=list(si.on_update)
                    )
                    changed = True
                new.append(inst)
            if changed:
                blk.instructions = new


# Tunables.
CFG = {
    "xt_bufs": 6,
    "ht_bufs": 13,
    "hp_bufs": 3,
    "tp_bufs": 2,
    "op_bufs": 2,
    "osb_bufs": 3,
    # 0=ACT 1=DVE per conv chunk (Pool/GPSIMD cannot read PSUM on TRN2 HW);
    # q=5 always runs the DVE tensor_scalar that injects the bias ones-row.
    "drain_order": (0, 1, 0, 1, 0, 1),
    "drain_out": 1,  # engine for the ops PSUM->SBUF copy (0=ACT 1=DVE)
    # Macro-tile batch sizes: small tiles at the start fill the DMA->PE
    # pipeline sooner (first conv waits on only a 128-row transpose), small
    # tiles at the end shorten the drain->fc->bias->store tail.
    "macros": (128, 128, 256, 512, 512, 512, 512, 512, 512, 256, 128, 128),
    "warmup_pe": 3,  # dependency-free PE matmuls at t~0 (p-state ramp)
    "pre_xpose": 2,  # input transposes emitted before the small const loads
}

_DR_RELU = mybir.ActivationFunctionType.Relu


def build_nc(bl=BL, split_waits=True):
    nc = bass.Bass("TRN2")
    xb = nc.dram_tensor("xb", [bl, PPAD], mybir.dt.bfloat16, kind="ExternalInput")
    wpk = nc.dram_tensor("wpack", [128, 1596], mybir.dt.bfloat16, kind="ExternalInput")
    fpk = nc.dram_tensor("fpack", [NCLS, NCLS], mybir.dt.float32, kind="ExternalInput")
    vpk = nc.dram_tensor("vpack", [128, 1], mybir.dt.float32, kind="ExternalInput")
    out = nc.dram_tensor("out", [bl, NCLS], mybir.dt.float32, kind="ExternalOutput")

    macros = [s for s in CFG["macros"]]
    assert sum(macros) * (bl // bl) == bl if bl == BL else True
    if sum(macros) != bl:  # fall back to uniform tiling for other shard sizes
        macros = [MACRO] * (bl // MACRO)

    with TileContext(nc) as tc:
        with (
            tc.tile_pool(name="const", bufs=1) as cp,
            tc.tile_pool(name="xt", bufs=CFG["xt_bufs"]) as xtp,
            tc.tile_pool(name="ht", bufs=CFG["ht_bufs"]) as htp,
            tc.tile_pool(name="osb", bufs=CFG["osb_bufs"]) as osp,
            tc.tile_pool(name="hps", bufs=CFG["hp_bufs"], space="PSUM") as hpp,
            tc.tile_pool(name="tps", bufs=CFG["tp_bufs"], space="PSUM") as tpp,
            tc.tile_pool(name="ops", bufs=CFG["op_bufs"], space="PSUM") as opp,
            tc.tile_pool(name="wup", bufs=1, space="PSUM") as wup,
        ):
            # PE warmup: matmuls on a zeroed tile, dependency-free so they
            # issue immediately and start the p-state ramp clock.
            if CFG["warmup_pe"]:
                zt = cp.tile([128, 128], mybir.dt.bfloat16)
                nc.gpsimd.memset(zt[:, :], 0)
                for w in range(CFG["warmup_pe"]):
                    wtp = wup.tile([128, NCLS], mybir.dt.float32, tag="wu")
                    nc.tensor.matmul(
                        wtp[:, :], zt[:, :], zt[:, 0:NCLS], start=True, stop=True
                    )

            wps = cp.tile([128, 1596], mybir.dt.bfloat16)
            nc.sync.dma_start(wps[:, :], wpk[:, :])
            w1s = wps[:, 0:768]
            w2s = wps[:, 768:1536]
            fcs = wps[:, 1536:1596]

            # First input transposes go ahead of the small const loads so the
            # conv pipeline fills as early as possible.
            starts = []
            acc = 0
            for nb in macros:
                starts.append(acc)
                acc += nb
            # xt tiles are allocated at the exact macro size so the xbar
            # transpose's destination AP is fully contiguous (a strided 3D
            # dest was flakily mis-written by the hardware xbar).
            def emit_xpose(pm):
                nb = macros[pm]
                xt = xtp.tile([128, 7, nb], mybir.dt.bfloat16, tag=f"xt{nb}")
                nc.sync.dma_start_transpose(
                    xt[:, :, :], xb[starts[pm] : starts[pm] + nb, :]
                )
                return xt

            pre_xt = {}
            for pm in range(min(CFG["pre_xpose"], len(macros))):
                pre_xt[pm] = emit_xpose(pm)

            fps = cp.tile([NCLS, NCLS], mybir.dt.float32)
            nc.sync.dma_start(fps[:, :], fpk[:, :])
            vps = cp.tile([128, 1], mybir.dt.float32)
            nc.sync.dma_start(vps[:, :], vpk[:, :])
            eyfs = fps[:, :]

            def emit_conv(m, nb, xt):
                """12 conv matmuls + 6 relu drains; returns ht tiles."""
                hts = []
                for q in range(6):
                    hp = hpp.tile([128, MACRO], mybir.dt.float32, tag="hp")
                    nc.tensor.matmul(
                        hp[:, 0:nb], w1s[:, 128 * q : 128 * (q + 1)], xt[:, q, :],
                        start=True, stop=False,
                    )
                    nc.tensor.matmul(
                        hp[:, 0:nb], w2s[:, 128 * q : 128 * (q + 1)], xt[:, q + 1, :],
                        start=False, stop=True,
                    )
                    ht = htp.tile([128, MACRO], mybir.dt.bfloat16, tag="ht")
                    if q == 5:
                        # relu + ones-row injection (row 87 <- 0 + 1.0) so the
                        # fc matmul's fct[727] row adds the bias.
                        nc.vector.tensor_scalar(
                            ht[:, 0:nb], hp[:, 0:nb], 0.0, vps[:, :],
                            op0=mybir.AluOpType.max, op1=mybir.AluOpType.add,
                        )
                    elif CFG["drain_order"][q] == 0:
                        nc.scalar.activation(ht[:, 0:nb], hp[:, 0:nb], _DR_RELU)
                    else:
                        nc.vector.tensor_scalar_max(ht[:, 0:nb], hp[:, 0:nb], 0.0)
                    hts.append(ht)
                return hts

            def emit_fc(nb, hts):
                """6 accumulating fc matmuls + PSUM->SBUF copy of out^T."""
                ops = opp.tile([NCLS, MACRO], mybir.dt.float32, tag="ops")
                for q in range(6):
                    nc.tensor.matmul(
                        ops[:, 0:nb], fcs[:, NCLS * q : NCLS * (q + 1)],
                        hts[q][:, 0:nb], start=(q == 0), stop=(q == 5),
                    )
                ot = osp.tile([NCLS, MACRO], mybir.dt.float32, tag="ot")
                if CFG["drain_out"] == 0:
                    nc.scalar.copy(ot[:, 0:nb], ops[:, 0:nb])
                else:
                    nc.vector.tensor_copy(ot[:, 0:nb], ops[:, 0:nb])
                return ot

            def emit_out(m, nb, ot):
                """PE transposes back to batch-major + one store DMA."""
                r0 = starts[m]
                nbc = nb // 128
                ob4 = osp.tile([128, (MACRO // 128) * NCLS], mybir.dt.float32, tag="ob4")
                for bc in range(nbc):
                    op2 = tpp.tile([128, NCLS], mybir.dt.float32, tag="tp")
                    nc.tensor.transpose(
                        op2[:, :], ot[:, bc * 128 : (bc + 1) * 128], eyfs[:, :]
                    )
                    if bc % 2 == 0:
                        nc.vector.tensor_copy(ob4[:, bc * NCLS : (bc + 1) * NCLS], op2[:, :])
                    else:
                        nc.scalar.copy(ob4[:, bc * NCLS : (bc + 1) * NCLS], op2[:, :])
                nc.sync.dma_start(
                    out[r0 : r0 + nb, :].rearrange("(b p) c -> p b c", p=128),
                    ob4[:, 0 : nbc * NCLS].rearrange("p (b c) -> p b c", c=NCLS),
                )

            # Two-deep software pipeline on the in-order PE queue: iteration m
            # emits conv_m, fc_{m-1}, out_{m-2}, so fc never waits on a drain
            # that was just issued and out-transposes never wait on the ot copy.
            fcq = []   # [(m, nb, hts)]
            outq = []  # [(m, nb, ot)]
            for m, nb in enumerate(macros):
                xt = pre_xt.pop(m) if m in pre_xt else emit_xpose(m)
                hts = emit_conv(m, nb, xt)
                fcq.append((m, nb, hts))
                if len(fcq) > 1:
                    fm, fnb, fhts = fcq.pop(0)
                    outq.append((fm, fnb, emit_fc(fnb, fhts)))
                if len(outq) > 1:
                    om, onb, oot = outq.pop(0)
                    emit_out(om, onb, oot)
            while fcq:
                fm, fnb, fhts = fcq.pop(0)
                outq.append((fm, fnb, emit_fc(fnb, fhts)))
            while outq:
                om, onb, oot = outq.pop(0)
                emit_out(om, onb, oot)
    if split_waits:
        _split_waits(nc)
    return nc


_CACHED = {}


def _get_nc(bl):
    if bl not in _CACHED:
        _CACHED[bl] = build_nc(bl)
    return _CACHED[bl]


def kernel(x, conv_w, fc_w, fc_b):
    x = np.ascontiguousarray(np.asarray(x, dtype=np.float32))
    conv_w = np.asarray(conv_w, dtype=np.float32)
    fc_w = np.asarray(fc_w, dtype=np.float32)
    fc_b = np.asarray(fc_b, dtype=np.float32)

    wpack, fpack, vpack = _host_packs(conv_w, fc_w, fc_b)
    xbig = _host_x(x)

    nc = _get_nc(BL)
    in_maps = []
    for c in range(NCORES):
        in_maps.append(
            {
                "xb": xbig[c * BL : (c + 1) * BL],
                "wpack": wpack,
                "fpack": fpack,
                "vpack": vpack,
            }
        )
    # The axon-proxied NeuronCores occasionally come up wedged
    # (NRT_EXEC_UNIT_UNRECOVERABLE) on the first execute after idle periods;
    # a retry on a fresh execute reliably recovers.
    last_err = None
    for _attempt in range(3):
        try:
            res = run_bass_kernel_spmd(nc, in_maps, core_ids=list(range(NCORES)))
            break
        except Exception as e:  # noqa: BLE001
            last_err = e
            if "UNRECOVERABLE" not in str(e) and "desynced" not in str(e):
                raise
    else:
        raise last_err
    out = np.concatenate([np.asarray(r["out"]) for r in res.results], axis=0)
    return out


if __name__ == "__main__":
    rng = np.random.default_rng(0)
    xs = rng.standard_normal((B, PIX), dtype=np.float32)
    cw = rng.standard_normal((3, 3), dtype=np.float32)
    fw = (rng.standard_normal((NCLS, 676)) * 0.05).astype(np.float32)
    fb = (rng.standard_normal((NCLS,)) * 0.05).astype(np.float32)
    res = kernel(xs, cw, fw, fb)
    print(res.shape, res.dtype)
